# revision 1
# baseline (speedup 1.0000x reference)
"""Causal self-attention (RoPE) Trainium2 Bass kernel, 8-core SPMD.

Sharding: core c = (batch b = c//2, head-group g = c%2); each core computes
4 heads of one batch element end-to-end, producing a partial [T, C] output;
the host sums the two head-group partials per batch.

Attention math (per core, per head): S^T[k, q] = K_rope^T Q_rope with
k-blocks on partitions (no on-device transposes; activations live
feature-major). Softmax needs only exp -- inputs are bounded -- with the
denominator coming free from an all-ones 65th column appended to V. The
P@V matmul is token-major: out[q=128part, d=65free] = P^T(lhsT) @
V_aug(rhs), costing 65 PE cycles per 128x128 tile instead of 128+, and
making the softmax normalization a native per-partition scalar broadcast
(tensor_scalar_mul) instead of a partition broadcast. P, V, Q, K are bf16
(1 cycle/row at any width on the PE; exact-causal diagonal tiles); a cheap
PE identity-permutation transpose restores feat-major pkq for the output
projection. Causal masking: full below-diagonal k-blocks need none; the
diagonal 512-block packs its four 128-tiles into two exp batches with
gpsimd affine_select zeroing the strictly-upper strips, and token-major PV
simply skips above-diagonal tiles.

Schedule: a flat pipeline over 8 blocks P(pair, q-slice), slice-major.
Each block's full S->exp stream is emitted one block EARLY (inside the
previous block) with its PV matmuls held as pend-thunks gated on the
block's PSUM accumulator allocation, so the ACT exp stream never drains
across an epilogue (finalize / transpose / outproj). QKV projection + RoPE
for slice q and the output projection for slice q-1 are distributed into
fixed slots chosen so every tensor is emitted before its first reader.

PSUM (8 banks): S-pair tiles [128,1024]x2bufs = 4, PV accumulators
[128,260]x2 = 2 (each: 2 q-blocks x 2 heads x 65, ONE start / ONE stop
per bank -- accumulation groups are bank-granular), shared scratch
[128,512]x2bufs = 2 (proj / rope-rot / vproj / outproj psum + transposes).
PSUM accumulation start=True marks its whole 2KB bank pending-zero, so
later sub-regions overwrite-on-first-write.
"""

import numpy as np

B, T, C = 4, 2048, 512
H_TOT, HD = 8, 64
HL = 4          # heads per core
NCORES = 8

_prog_cache = {}
LAST_EXEC_NS = None
LAST_RESULTS = None


def _build_program(t=T):
    import concourse.tile as tile
    from concourse import bacc, mybir

    f32 = mybir.dt.float32
    f32r = mybir.dt.float32r
    bf16 = mybir.dt.bfloat16
    Exp = mybir.ActivationFunctionType.Exp

    nt = t // 512      # 512-wide token slices
    nb = t // 128      # 128-wide token blocks

    nc = bacc.Bacc("TRN2", target_bir_lowering=False, debug=False,
                   enable_asserts=False, num_devices=NCORES)

    xT = nc.dram_tensor("xT", [C, t], bf16, kind="ExternalInput").ap()
    wqkT = nc.dram_tensor("wqkT", [C, 512], bf16, kind="ExternalInput").ap()
    wvT = nc.dram_tensor("wvT", [C, 256], bf16, kind="ExternalInput").ap()
    woT = nc.dram_tensor("woT", [256, C], bf16, kind="ExternalInput").ap()
    cos2T = nc.dram_tensor("cos2T", [128, t], bf16, kind="ExternalInput").ap()
    sin2T = nc.dram_tensor("sin2T", [128, t], bf16, kind="ExternalInput").ap()
    r2T = nc.dram_tensor("r2T", [128, 128], f32r, kind="ExternalInput").ap()
    eyeT = nc.dram_tensor("eyeT", [128, 128], bf16, kind="ExternalInput").ap()
    y = nc.dram_tensor("y", [t, C], bf16, kind="ExternalOutput").ap()

    with tile.TileContext(nc) as tc:
        with tc.tile_pool(name="persist", bufs=1) as pp, \
             tc.tile_pool(name="ptiles", bufs=4) as ppool, \
             tc.tile_pool(name="psum", bufs=1, space="PSUM") as ps:

            # ---- constants & weights ----
            r2_t = pp.tile([128, 128], f32r, tag="r2")
            eye_t = pp.tile([128, 128], bf16, tag="eye")
            wq_t = [pp.tile([128, 512], bf16, tag=f"wq{i}", name=f"wq{i}")
                    for i in range(4)]
            wv_t = [pp.tile([128, 256], bf16, tag=f"wv{i}", name=f"wv{i}")
                    for i in range(4)]
            wo_t = [pp.tile([128, 512], bf16, tag=f"wo{i}", name=f"wo{i}")
                    for i in range(2)]
            cos_sl = [pp.tile([128, 512], bf16, tag=f"cos{s_}",
                              name=f"cos{s_}") for s_ in range(nt)]
            sin_sl = [pp.tile([128, 512], bf16, tag=f"sin{s_}",
                              name=f"sin{s_}") for s_ in range(nt)]

            # ---- persistent activations ----
            # qk_sl[i][ts]: i=0,1 -> q head-pairs (01, 23); i=2,3 -> k pairs
            qk_sl = [[pp.tile([128, 512], bf16, tag=f"qkr{i}_{s_}",
                              name=f"qkr{i}_{s_}") for s_ in range(nt)]
                     for i in range(4)]
            # v_aug[tb]: [128 k-tok, 4 heads x (64 v-feats + ones)] bf16
            v_aug = [pp.tile([128, 260], bf16, tag=f"va{i}", name=f"va{i}")
                     for i in range(nb)]
            # pkq[fc]: attention output, feat-major [128 feats, t] bf16
            pkq = [pp.tile([128, t], bf16, tag=f"pkq{i}", name=f"pkq{i}")
                   for i in range(2)]

            def load_x(ts):
                # SP queue: Pool carries the rope/mask elementwise work, so
                # keep bulk input DMAs off it
                sl = slice(ts * 512, (ts + 1) * 512)
                x_ts = [pp.tile([128, 512], bf16, tag=f"xs{cc}", bufs=2,
                                name=f"xs{cc}") for cc in range(4)]
                for cc in range(4):
                    nc.sync.dma_start(out=x_ts[cc],
                                      in_=xT[cc * 128:(cc + 1) * 128, sl])
                nc.sync.dma_start(out=cos_sl[ts], in_=cos2T[:, sl])
                nc.sync.dma_start(out=sin_sl[ts], in_=sin2T[:, sl])
                return x_ts

            def emit_proj_fb(ts, fb, x_ts, evac=None):
                psum = ps.tile([128, 512], f32, tag="sd", bufs=2)
                for cc in range(4):
                    nc.tensor.matmul(
                        psum[:],
                        wq_t[cc][:, fb * 128:(fb + 1) * 128],
                        x_ts[cc][:],
                        start=(cc == 0), stop=(cc == 3))
                qkp = pp.tile([128, 512], f32r, tag="qkp", bufs=2)
                if evac == "act":
                    # startup only: ACT is idle before the first exps and
                    # the in-order DVE otherwise serializes the early rope
                    # chains behind finalize/vproj work
                    nc.scalar.copy(out=qkp[:], in_=psum[:])
                else:
                    nc.vector.tensor_copy(out=qkp[:], in_=psum[:])
                rps = ps.tile([128, 512], f32, tag="sd", bufs=2)
                nc.tensor.matmul(rps[:], r2_t[:], qkp[:],
                                 start=True, stop=True)
                tmp = pp.tile([128, 512], f32r, tag="ropetmp", bufs=2)
                nc.vector.tensor_mul(out=tmp[:], in0=rps[:],
                                     in1=sin_sl[ts][:])
                tmp2 = pp.tile([128, 512], f32r, tag="ropetmp2", bufs=2)
                nc.gpsimd.tensor_mul(out=tmp2[:], in0=qkp[:],
                                     in1=cos_sl[ts][:])
                with nc.allow_low_precision(reason="bf16 qk"):
                    nc.gpsimd.tensor_add(out=qk_sl[fb][ts][:],
                                         in0=tmp[:], in1=tmp2[:])

            def emit_vproj(ts, tbl, x_ts, evac=None):
                tb = ts * 4 + tbl
                vpsum = ps.tile([128, 256], f32, tag="sd", bufs=2)
                for cc in range(4):
                    nc.tensor.matmul(
                        vpsum[:],
                        x_ts[cc][:, tbl * 128:(tbl + 1) * 128],
                        wv_t[cc][:],
                        start=(cc == 0), stop=(cc == 3))
                with nc.allow_low_precision(reason="bf16 v"):
                    if evac == "act":
                        nc.scalar.copy(
                            out=v_aug[tb].rearrange(
                                "p (h c) -> p h c", h=4)[:, :, 0:64],
                            in_=vpsum.rearrange("p (h c) -> p h c", h=4))
                    else:
                        nc.vector.tensor_copy(
                            out=v_aug[tb].rearrange(
                                "p (h c) -> p h c", h=4)[:, :, 0:64],
                            in_=vpsum.rearrange("p (h c) -> p h c", h=4))

            def pv_region(pvt, qb, hh):
                return pvt[qb // 2][:, (qb % 2) * 130 + hh * 65:
                                    (qb % 2) * 130 + hh * 65 + 65]

            # pending-PV thunks, tagged by block: a unit's PV matmuls
            # are emitted only after a later exp is issued, so the PE always
            # has PV work queued behind ACT. A thunk may only run once its
            # block's PV accumulator exists (blk["pvt"]); the FIFO stops at
            # the first not-yet-ready block, which also keeps the
            # one-start/one-stop per-bank discipline intact.
            pv_pend = []

            def flush_pv():
                while pv_pend and pv_pend[0][0]["pvt"] is not None:
                    pv_pend.pop(0)[1]()

            def stream_fulls(blk, hh, kp_lo, kp_hi):
                """S^T -> exp -> token-major PV for full k-block pairs
                [kp_lo, kp_hi) of one head's q-slice.

                PSUM accumulation groups are bank-granular (2KB zero
                regions): exactly ONE start=True per pv bank per block --
                the first matmul into the bank (hh=0, kb=0) -- which marks
                the whole bank pending-zero so later sub-regions
                overwrite-on-first-write; ONE stop=True on the bank's
                chronologically last matmul (hh=1 diag).
                """
                qs, pair = blk["qs"], blk["pair"]
                half = hh * 64
                hl = 2 * pair + hh
                first_hh = hh == 0
                qsl_t = qk_sl[pair][qs]
                qh = qsl_t[half:half + 64, :]

                def kslc(kb):
                    return qk_sl[2 + pair][kb // 4][
                        half:half + 64, (kb % 4) * 128:(kb % 4 + 1) * 128]

                def emit_pv(pT, col_off, kb):
                    for qb in range(4):
                        nc.tensor.matmul(
                            pv_region(blk["pvt"], qb, hh),
                            pT[:, col_off + qb * 128:col_off + (qb + 1) * 128],
                            v_aug[kb][:, hl * 65:(hl + 1) * 65],
                            start=(first_hh and kb == 0 and qb % 2 == 0),
                            stop=False, skip_group_check=True)

                for kp in range(kp_lo, kp_hi):
                    ka, kb_ = 2 * kp, 2 * kp + 1
                    spsum = ps.tile([128, 1024], f32, tag="s", bufs=2)
                    nc.tensor.matmul(
                        spsum[:, 0:512], kslc(ka), qh,
                        start=True, stop=True)
                    nc.tensor.matmul(
                        spsum[:, 512:1024], kslc(kb_), qh,
                        start=True, stop=True)
                    pT = ppool.tile([128, 1024], bf16, tag="pT", bufs=18)
                    nc.scalar.activation(out=pT[:], in_=spsum[:], func=Exp)
                    flush_pv()
                    pv_pend.append((blk, lambda pT=pT, ka=ka, kb_=kb_: (
                        emit_pv(pT, 0, ka), emit_pv(pT, 512, kb_))))

            def stream_diag(blk, hh):
                qs, pair = blk["qs"], blk["pair"]
                half = hh * 64
                hl = 2 * pair + hh
                first_hh = hh == 0
                last_hh = hh == 1
                qsl_t = qk_sl[pair][qs]

                def kslc(kb):
                    return qk_sl[2 + pair][kb // 4][
                        half:half + 64, (kb % 4) * 128:(kb % 4 + 1) * 128]

                # diagonal 512-block: j0 (q 0:512) + j1 (q 128:512) in one
                # exp; j2 + j3 (widened to q 256:512 for f32r speed) in a
                # second. affine_select zeroes the strictly-upper strips.
                j0 = 4 * qs
                spA = ps.tile([128, 1024], f32, tag="s", bufs=2)
                nc.tensor.matmul(
                    spA[:, 0:512], kslc(j0),
                    qsl_t[half:half + 64, 0:512], start=True, stop=True)
                nc.tensor.matmul(
                    spA[:, 512:896], kslc(j0 + 1),
                    qsl_t[half:half + 64, 128:512], start=True, stop=True)
                pTa = ppool.tile([128, 1024], bf16, tag="pT", bufs=18)
                nc.scalar.activation(out=pTa[:, 0:896], in_=spA[:, 0:896],
                                     func=Exp)
                flush_pv()
                for c0 in (0, 512):
                    nc.gpsimd.affine_select(
                        out=pTa[:, c0:c0 + 128], in_=pTa[:, c0:c0 + 128],
                        compare_op=mybir.AluOpType.is_ge, fill=0.0,
                        base=0, channel_multiplier=-1, pattern=[[1, 128]])
                spB = ps.tile([128, 512], f32, tag="s", bufs=2)
                nc.tensor.matmul(
                    spB[:, 0:256], kslc(j0 + 2),
                    qsl_t[half:half + 64, 256:512], start=True, stop=True)
                nc.tensor.matmul(
                    spB[:, 256:384], kslc(j0 + 3),
                    qsl_t[half:half + 64, 384:512], start=True, stop=True)
                pTb = ppool.tile([128, 512], bf16, tag="pTd", bufs=8)
                nc.scalar.activation(out=pTb[:, 0:384], in_=spB[:, 0:384],
                                     func=Exp)
                flush_pv()
                nc.gpsimd.affine_select(
                    out=pTb[:, 0:128], in_=pTb[:, 0:128],
                    compare_op=mybir.AluOpType.is_ge, fill=0.0,
                    base=0, channel_multiplier=-1, pattern=[[1, 128]])
                nc.gpsimd.affine_select(
                    out=pTb[:, 256:384], in_=pTb[:, 256:384],
                    compare_op=mybir.AluOpType.is_ge, fill=0.0,
                    base=0, channel_multiplier=-1, pattern=[[1, 128]])

                def emit_diag_pv():
                    # diagonal PV, skipping above-diagonal 128-tiles.
                    # bank0 (qb 0,1) last touch = j1@qb1; bank1 = j3@qb3
                    pvt = blk["pvt"]
                    va = v_aug
                    for qb in range(4):          # j0 covers q 0:512
                        nc.tensor.matmul(
                            pv_region(pvt, qb, hh),
                            pTa[:, qb * 128:(qb + 1) * 128],
                            va[j0][:, hl * 65:(hl + 1) * 65],
                            start=(first_hh and qs == 0 and qb % 2 == 0),
                            stop=False, skip_group_check=True)
                    for qb in range(1, 4):       # j1 covers q 128:512
                        nc.tensor.matmul(
                            pv_region(pvt, qb, hh),
                            pTa[:, 512 + (qb - 1) * 128:512 + qb * 128],
                            va[j0 + 1][:, hl * 65:(hl + 1) * 65],
                            start=False, stop=(last_hh and qb == 1),
                            skip_group_check=True)
                    for qb in range(2, 4):       # j2 covers q 256:512
                        nc.tensor.matmul(
                            pv_region(pvt, qb, hh),
                            pTb[:, (qb - 2) * 128:(qb - 1) * 128],
                            va[j0 + 2][:, hl * 65:(hl + 1) * 65],
                            start=False, stop=False, skip_group_check=True)
                    nc.tensor.matmul(            # j3 covers q 384:512
                        pv_region(pvt, 3, hh),
                        pTb[:, 256:384],
                        va[j0 + 3][:, hl * 65:(hl + 1) * 65],
                        start=False, stop=last_hh, skip_group_check=True)

                pv_pend.append((blk, emit_diag_pv))

            def finalize_half(blk, ti):
                # per-q softmax denominators sit at col 64 of each 65-group;
                # reciprocal + per-partition scalar broadcast normalize
                qs, pair, pvt = blk["qs"], blk["pair"], blk["pvt"]
                attn_sb = blk["asb"]
                gv = pvt[ti].rearrange("p (g s) -> p g s", s=65)
                rc = pp.tile([128, 4], f32, tag="rc", bufs=2)
                with nc.allow_low_precision(reason="f32 recip"):
                    nc.vector.reciprocal(
                        out=rc.rearrange("p (g o) -> p g o", o=1),
                        in_=gv[:, :, 64:65])
                for g in range(4):
                    qb = ti * 2 + g // 2
                    hh = g % 2
                    c0 = pair * 128 + hh * 64
                    with nc.allow_low_precision(reason="bf16 attn"):
                        nc.vector.tensor_scalar_mul(
                            attn_sb[qb][:, c0:c0 + 64],
                            pvt[ti][:, g * 65:g * 65 + 64],
                            rc[:, g:g + 1])

            def emit_transpose(qs, qb, attn_sb, fc, tag="sd", bufs=2,
                               evac=None):
                # [128 tok, 128 feat] PE transpose into feat-major pkq
                # (identity-permutation matmul, bf16); fc == source pair
                tp = ps.tile([128, 128], bf16, tag=tag, bufs=bufs,
                             name=f"tp{fc}")
                nc.tensor.matmul(
                    tp[:], attn_sb[qb][:, fc * 128:(fc + 1) * 128],
                    eye_t[:], is_transpose=True)
                dst = pkq[fc][:, qs * 512 + qb * 128:
                              qs * 512 + (qb + 1) * 128]
                if evac == "act":
                    nc.scalar.copy(out=dst, in_=tp[:])
                else:
                    nc.vector.tensor_copy(out=dst, in_=tp[:])

            def emit_outproj(qs, tbl, evac=None):
                ypsum = ps.tile([128, 512], f32, tag="sd", bufs=2)
                for fc in range(2):
                    nc.tensor.matmul(
                        ypsum[:],
                        pkq[fc][:, (qs * 4 + tbl) * 128:
                                (qs * 4 + tbl + 1) * 128],
                        wo_t[fc][:],
                        start=(fc == 0), stop=(fc == 1))
                ysb = pp.tile([128, 512], bf16, tag="ysb", bufs=3)
                with nc.allow_low_precision(reason="bf16 y partial"):
                    if evac == "act":
                        nc.scalar.copy(out=ysb[:], in_=ypsum[:])
                    else:
                        nc.vector.tensor_copy(out=ysb[:], in_=ypsum[:])
                dq = (nc.sync, nc.scalar, nc.gpsimd, nc.sync)[tbl] \
                    if evac == "act" else nc.sync
                dq.dma_start(
                    out=y[(qs * 4 + tbl) * 128:(qs * 4 + tbl + 1) * 128, :],
                    in_=ysb[:])

            def alloc_pvt():
                return [ps.tile([128, 260], f32, tag=f"pv{i}",
                                bufs=1, name=f"pv{i}") for i in range(2)]

            def alloc_asb():
                return [pp.tile([128, 256], bf16, tag=f"asb{qb}", bufs=2,
                                name=f"asb{qb}") for qb in range(4)]

            # ---- prologue: interleave x(0) with wq so the first projection
            # matmuls start ASAP; bulky constants follow
            sl0 = slice(0, 512)
            x_of = {}
            x_of[0] = [pp.tile([128, 512], bf16, tag=f"xs{cc}", bufs=2,
                               name=f"xs{cc}") for cc in range(4)]
            # fan the first x slice across all four issuing queues -- DMA
            # transfer time serializes per queue, and x(0) heads the
            # critical path
            # pair each x0 chunk with its wq tile on the same queue so
            # the 4th projection matmul isn't serialized behind all of x0
            x0q = [nc.sync, nc.scalar, nc.gpsimd, nc.sync]
            for cc in range(4):
                x0q[cc].dma_start(out=x_of[0][cc],
                                  in_=xT[cc * 128:(cc + 1) * 128, sl0])
                x0q[cc].dma_start(out=wq_t[cc],
                                  in_=wqkT[cc * 128:(cc + 1) * 128, :])
            nc.scalar.dma_start(out=r2_t, in_=r2T)
            nc.gpsimd.dma_start(out=cos_sl[0], in_=cos2T[:, 0:512])
            nc.gpsimd.dma_start(out=sin_sl[0], in_=sin2T[:, 0:512])
            wvq = [nc.sync, nc.sync, nc.gpsimd, nc.gpsimd]
            for i in range(4):
                wvq[i].dma_start(out=wv_t[i],
                                 in_=wvT[i * 128:(i + 1) * 128, :])
            nc.gpsimd.dma_start(out=eye_t, in_=eyeT)
            # x(1) split early across the not-yet-busy ACT/DVE queues
            x_of[1] = [pp.tile([128, 512], bf16, tag=f"xs{cc}", bufs=2,
                               name=f"xs{cc}") for cc in range(4)]
            sl1 = slice(512, 1024)
            x1q = [nc.scalar, nc.scalar, nc.gpsimd, nc.gpsimd]
            for cc in range(4):
                x1q[cc].dma_start(out=x_of[1][cc],
                                  in_=xT[cc * 128:(cc + 1) * 128, sl1])
            nc.scalar.dma_start(out=cos_sl[1], in_=cos2T[:, sl1])
            nc.scalar.dma_start(out=sin_sl[1], in_=sin2T[:, sl1])
            # ones columns of v_aug (persistent; vproj never touches them)
            for tb in range(nb):
                nc.gpsimd.memset(
                    v_aug[tb].rearrange("p (h c) -> p h c", h=4)[:, :, 64:65],
                    1.0)
            emit_proj_fb(0, 0, x_of[0])
            emit_proj_fb(0, 2, x_of[0])

            # ---- flat block pipeline over P(pair, qs) in slice-major
            # order, with one-block exp lookahead across every boundary so
            # ACT never drains while an epilogue (finalize / transpose /
            # outproj chain) runs.
            blocks = [{"qs": b // 2, "pair": b % 2, "pvt": None,
                       "la_done": False}
                      for b in range(2 * nt)]
            asb_of = {}

            def sched_start(b):
                qs, pair = blocks[b]["qs"], blocks[b]["pair"]
                if pair == 1 and qs + 2 < nt:
                    x_of[qs + 2] = load_x(qs + 2)
                if b == 1:
                    for i in range(2):
                        nc.sync.dma_start(out=wo_t[i],
                                          in_=woT[i * 128:(i + 1) * 128, :])

            # slot table: every proj / vproj / outproj group placed at
            # the latest emission point that still precedes its first
            # reader under full-block lookahead (stream_block(k) runs
            # inside emit_block(k-1); PV flushes of block k begin after
            # slot (k, 1))
            def _fb(q, i):
                return lambda: emit_proj_fb(q, i, x_of[q])

            def _vp(q, t):
                return lambda: emit_vproj(q, t, x_of[q])

            def _op(q, t):
                return lambda: emit_outproj(q, t)

            SLOTS = {
                (0, 0): [_fb(1, 1), _vp(1, 0)],
                (0, 1): [_fb(1, 3), _vp(1, 1)],
                (1, 0): [_vp(1, 2)],
                (1, 1): [_vp(1, 3)],
                (2, 0): [_fb(2, 0), _vp(2, 0), _op(0, 0)],
                (2, 1): [_fb(2, 2), _vp(2, 1), _op(0, 1)],
                (3, 0): [_fb(2, 1), _vp(2, 2), _op(0, 2)],
                (3, 1): [_fb(2, 3), _vp(2, 3), _op(0, 3)],
                (4, 0): [_fb(3, 0), _vp(3, 0), _op(1, 0)],
                (4, 1): [_fb(3, 2), _vp(3, 1), _op(1, 1)],
                (5, 0): [_fb(3, 1), _vp(3, 2), _op(1, 2)],
                (5, 1): [_fb(3, 3), _vp(3, 3), _op(1, 3)],
                (6, 0): [_op(2, 0), _op(2, 1)],
                (6, 1): [_op(2, 2)],
                (7, 0): [_op(2, 3)], (7, 1): [],
            }

            def sched_slot(b, hh):
                for fn in SLOTS.get((b, hh), []):
                    fn()

            def sched_post(b):
                pass

            # block 0 streams in the prologue, vprojs interleaved between
            # its two diag streams so the first exp isn't queued behind them
            blocks[0]["asb"] = asb_of[0] = alloc_asb()
            blocks[0]["pvt"] = alloc_pvt()
            stream_diag(blocks[0], 0)
            emit_proj_fb(0, 1, x_of[0])
            emit_proj_fb(0, 3, x_of[0])
            for tbl in range(4):
                emit_vproj(0, tbl, x_of[0])
            stream_diag(blocks[0], 1)
            blocks[0]["la_done"] = True
            emit_proj_fb(1, 0, x_of[1])
            emit_proj_fb(1, 2, x_of[1])

            def stream_block(blk):
                if blk["qs"] >= 1:
                    stream_fulls(blk, 0, 0, 2 * blk["qs"])
                stream_diag(blk, 0)
                if blk["qs"] >= 1:
                    stream_fulls(blk, 1, 0, 2 * blk["qs"])
                stream_diag(blk, 1)

            for b in range(2 * nt):
                blk = blocks[b]
                qs, pair = blk["qs"], blk["pair"]
                sched_start(b)
                if pair == 0 and qs not in asb_of:
                    asb_of[qs] = alloc_asb()
                blk["asb"] = asb_of[qs]
                if blk["pvt"] is None:
                    blk["pvt"] = alloc_pvt()
                if not blk["la_done"]:
                    stream_block(blk)
                if b == 2 * nt - 1:
                    # last block: no following stream will flush its pends
                    # progressively -- release them now so the PVs overlap
                    # its still-running exps instead of draining serially
                    flush_pv()
                # lookahead: next block's ENTIRE stream (S/exp only -- its
                # PV thunks stay pend-gated until that block's PSUM
                # accumulators exist; each of its exps progressively
                # flushes THIS block's PVs), so neither engine drains at
                # the boundary. Slots come after, filling the PE behind
                # the ACT-paced stream.
                if b + 1 < 2 * nt and not blocks[b + 1]["la_done"]:
                    nblk = blocks[b + 1]
                    stream_block(nblk)
                    nblk["la_done"] = True
                if b == 0:
                    # qs-0 blocks are tiny: also stream P(0,1) now so ACT
                    # rides through epilogues 0 and 1 without draining
                    stream_block(blocks[2])
                    blocks[2]["la_done"] = True
                sched_slot(b, 0)
                sched_slot(b, 1)
                flush_pv()
                last = b == 2 * nt - 1
                finalize_half(blk, 0)
                for qb in (0, 1):
                    # final epilogue: ride the freed pv banks so the four
                    # outproj psums get "sd" to themselves, and chase each
                    # transpose with its outproj immediately
                    if last:
                        emit_transpose(qs, qb, blk["asb"], fc=pair,
                                       tag="pv0", bufs=1)
                        emit_outproj(qs, qb, evac="act")
                    else:
                        emit_transpose(qs, qb, blk["asb"], fc=pair)
                finalize_half(blk, 1)
                for qb in (2, 3):
                    if last:
                        emit_transpose(qs, qb, blk["asb"], fc=pair,
                                       tag="pv1", bufs=1)
                        emit_outproj(qs, qb, evac="act")
                    else:
                        emit_transpose(qs, qb, blk["asb"], fc=pair)
                sched_post(b)

    nc.compile()
    return nc


def _rot_mats():
    r = np.zeros((64, 64), np.float32)
    r[np.arange(32), np.arange(32) + 32] = -1.0
    r[np.arange(32, 64), np.arange(32)] = 1.0
    r2 = np.zeros((128, 128), np.float32)
    r2[:64, :64] = r
    r2[64:, 64:] = r
    return np.ascontiguousarray(r2.T)


def _preprocess(x, cos, sin, W_qkv, W_out, t=T):
    import ml_dtypes
    bf = ml_dtypes.bfloat16
    r2T = _rot_mats()
    cosT = np.ascontiguousarray(cos[:t].T.astype(bf))
    sinT = np.ascontiguousarray(sin[:t].T.astype(bf))
    cos2 = np.concatenate([cosT, cosT], 0)
    sin2 = np.concatenate([sinT, sinT], 0)
    eye = np.eye(128, dtype=bf)

    in_maps = []
    for c in range(NCORES):
        b, g = divmod(c, 2)
        wq = W_qkv[g * 256:(g + 1) * 256] * 0.125
        wk = W_qkv[512 + g * 256:512 + (g + 1) * 256]
        wv = W_qkv[1024 + g * 256:1024 + (g + 1) * 256]
        in_maps.append({
            "xT": np.ascontiguousarray(x[b, :t].T.astype(bf)),
            "wqkT": np.ascontiguousarray(
                np.concatenate([wq, wk], 0).T.astype(bf)),
            "wvT": np.ascontiguousarray(wv.T.astype(bf)),
            "woT": np.ascontiguousarray(
                W_out.T[g * 256:(g + 1) * 256].astype(bf)),
            "cos2T": cos2, "sin2T": sin2, "r2T": r2T,
            "eyeT": eye,
        })
    return in_maps


def kernel(x, cos, sin, W_qkv, W_out, _trace=False):
    global LAST_EXEC_NS, LAST_RESULTS
    from concourse.bass_utils import run_bass_kernel_spmd

    x = np.asarray(x); cos = np.asarray(cos); sin = np.asarray(sin)
    W_qkv = np.asarray(W_qkv); W_out = np.asarray(W_out)

    if T not in _prog_cache:
        _prog_cache[T] = _build_program(T)
    nc = _prog_cache[T]

    in_maps = _preprocess(x, cos, sin, W_qkv, W_out)
    try:
        res = run_bass_kernel_spmd(nc, in_maps, list(range(NCORES)),
                                   trace=_trace)
    except ModuleNotFoundError:
        res = run_bass_kernel_spmd(nc, in_maps, list(range(NCORES)),
                                   trace=False)
    LAST_EXEC_NS = res.exec_time_ns
    LAST_RESULTS = res
    out = np.empty((B, T, C), np.float32)
    for b in range(B):
        out[b] = (res.results[2 * b]["y"].astype(np.float32)
                  + res.results[2 * b + 1]["y"].astype(np.float32))
    return out

